# revision 1
# baseline (speedup 1.0000x reference)
"""FFTConv2d kernel for trn2, 8 NeuronCores.

Math: reference einsum 'bchw,oihw->bohw' factorizes:
  Y[b,o] = conv_full(sum_c x[b,c], sum_i w[o,i])[1:-1,1:-1] + bias[o]
i.e. a single-channel 3x3 "same" convolution (flipped kernel) per (b,o).

Per core (2 batches):
  1. DMA x slice in as bf16 hi/lo pair (exact fp32 split), packed so each
     slice is one contiguous DMA; partitions=(b,c).
  2. Channel-sum via PE matmul with ones-indicator lhsT -> PSUM [6, n]
     (3 replicated copies per batch), accumulating hi+lo passes.
  3. Copy PSUM -> padded staging SBUF [6, 34*130] (row stride 130, zero
     borders), rounding to fp32r.
  4. Build P3 [8, 34*130]: partition (b,g) = staging col-shifted by (2-g);
     one contiguous SBUF->SBUF DMA each. Partitions (b,3) hold ones (bias).
  5. Conv: per 3-row output chunk, 3 accumulating fp32r matmuls (one per
     kernel row j) with rhs offset (2-j)*130 into P3 -> PSUM [128, 3, 130];
     all (b,o) images at once; bias rides the j=0 matmul's ones row.
  6. Copy PSUM -> Y SBUF (dropping the 2 pad columns per 130-row),
     DMA Y -> HBM.
Processed in NS row-slices for DMA/compute overlap.
"""

import os
import sys
from functools import lru_cache

import numpy as np

for _p in ("/opt/trn_rl_repo", "/root/.axon_site/_ro/trn_rl_repo"):
    if os.path.isdir(_p) and _p not in sys.path:
        sys.path.insert(0, _p)

import ml_dtypes

B, CIN, COUT, H, W = 16, 64, 64, 128, 128
N_CORES = 8
BPC = B // N_CORES  # batches per core = 2
NS = 4  # row slices per core
SH = H // NS  # rows per slice = 32
WROW = W + 2  # padded row stride = 130
PWIN = SH * WROW  # conv output window per slice = 4160
P3LEN = PWIN + 2 * WROW  # P3 length = 4420
SPLEN = P3LEN + 2  # staging length = 4422
NPART = BPC * CIN  # 128 input partitions (b, c)
NOUT = BPC * COUT  # 128 output partitions (b, o)
RMAX = SH + 2


def _slice_rows(s):
    h0 = max(0, SH * s - 1)
    he = min(H, SH * s + SH + 1)
    return h0, he


# packed input layout: per slice [hi rows | lo rows], contiguous
_SLICE_OFF = []
_off = 0
for _s in range(NS):
    _h0, _he = _slice_rows(_s)
    _SLICE_OFF.append(_off)
    _off += 2 * (_he - _h0) * W
XPACK_LEN = _off


@lru_cache(maxsize=1)
def _build():
    import concourse.bacc as bacc
    import concourse.mybir as mybir
    import concourse.tile as tile
    from concourse.ap import AP

    f32 = mybir.dt.float32
    f32r = mybir.dt.float32r
    bf16 = mybir.dt.bfloat16

    nc = bacc.Bacc("TRN2", target_bir_lowering=False, debug=False, num_devices=N_CORES)

    xp = nc.dram_tensor("xpack", [NPART, XPACK_LEN], bf16, kind="ExternalInput")
    ones_cs = nc.dram_tensor("ones_cs", [NPART, BPC * 3], bf16, kind="ExternalInput")
    wb = nc.dram_tensor("wb", [BPC * 9 + 1, NOUT], f32r, kind="ExternalInput")
    ones_p = nc.dram_tensor("ones_p", [1, PWIN], f32r, kind="ExternalInput")
    y = nc.dram_tensor("y", [NOUT, H * W], f32, kind="ExternalOutput")

    with tile.TileContext(nc) as tc:
        with (
            tc.tile_pool(name="xin", bufs=4) as xin_pool,
            tc.tile_pool(name="sp", bufs=1) as sp_pool,
            tc.tile_pool(name="pbuf", bufs=1) as p_pool,
            tc.tile_pool(name="yout", bufs=2) as y_pool,
            tc.tile_pool(name="consts", bufs=1) as c_pool,
            tc.tile_pool(name="cs_ps", bufs=4, space="PSUM") as cs_psum,
            tc.tile_pool(name="cv_ps", bufs=4, space="PSUM") as cv_psum,
        ):
            ones_t = c_pool.tile([NPART, BPC * 3], bf16, tag="ones_cs")
            nc.scalar.dma_start(out=ones_t[:, :], in_=ones_cs.ap()[:, :])
            wb_t = c_pool.tile([BPC * 9 + 1, NOUT], f32r, tag="wb")
            nc.scalar.dma_start(out=wb_t[:, :], in_=wb.ap()[:, :])

            # rotating staging + P3 + P9 buffers (zero borders persist)
            NBUF = 2
            NBUF9 = 3
            spbufs = []
            p9bufs = []
            for pi in range(NBUF):
                sp = sp_pool.tile([BPC * 3, SPLEN], f32r, tag=f"SP{pi}")
                spt0 = sp.tensor
                nc.vector.memset(
                    AP(tensor=spt0, offset=WROW - 1,
                       ap=[[SPLEN, BPC * 3], [WROW, RMAX], [1, 2]]).bitcast(f32),
                    0.0,
                )
                nc.vector.memset(sp[:, 0:WROW].bitcast(f32), 0.0)
                nc.vector.memset(sp[:, SPLEN - 1 : SPLEN].bitcast(f32), 0.0)
                spbufs.append(sp)
            for pi in range(NBUF9):
                p9 = p_pool.tile([BPC * 9 + 1, PWIN], f32r, tag=f"P9{pi}")
                nc.sync.dma_start(
                    out=p9[BPC * 9 : BPC * 9 + 1, :], in_=ones_p.ap()[0:1, :]
                )
                p9bufs.append(p9)

            def emit_in(s):
                h0, he = _slice_rows(s)
                ncols = (he - h0) * W
                xin = xin_pool.tile([NPART, 2 * RMAX * W], bf16, tag="xin")
                o = _SLICE_OFF[s]
                if s == 0:
                    # finer pieces so the first matmuls start sooner
                    for a0, a1 in ((0, 2048), (2048, ncols)):
                        nc.scalar.dma_start(
                            out=xin[:, a0:a1], in_=xp.ap()[:, o + a0 : o + a1]
                        )
                    for a0, a1 in ((0, 2048), (2048, ncols)):
                        nc.scalar.dma_start(
                            out=xin[:, ncols + a0 : ncols + a1],
                            in_=xp.ap()[:, o + ncols + a0 : o + ncols + a1],
                        )
                else:
                    nc.scalar.dma_start(
                        out=xin[:, :ncols], in_=xp.ap()[:, o : o + ncols]
                    )
                    nc.scalar.dma_start(
                        out=xin[:, ncols : 2 * ncols],
                        in_=xp.ap()[:, o + ncols : o + 2 * ncols],
                    )
                return xin

            def emit_cs_and_p(s, xin):
                hbase = SH * s - 1  # staging v-row 0 = image row hbase
                h0, he = _slice_rows(s)
                ncols = (he - h0) * W
                sp = spbufs[s % NBUF]
                spt = sp.tensor
                p9 = p9bufs[s % NBUF9]

                if s == NS - 1:
                    # bottom border: zero staging rows beyond image row 127
                    vz = (H - hbase) * WROW
                    nc.vector.memset(sp[:, vz:SPLEN].bitcast(f32), 0.0)

                # channel sum: ones^T @ [xhi; xlo], PSUM -> padded staging
                nchunks = (ncols + 511) // 512
                for ci in range(nchunks):
                    c0 = ci * 512
                    cn = min(512, ncols - c0)
                    nrows = cn // W
                    ps = cs_psum.tile([BPC * 3, 4, W], f32, tag="cs")
                    nc.tensor.matmul(
                        ps[:, :nrows, :],
                        ones_t[:, :],
                        xin[:, c0 : c0 + cn],
                        start=True,
                        stop=False,
                    )
                    nc.tensor.matmul(
                        ps[:, :nrows, :],
                        ones_t[:, :],
                        xin[:, ncols + c0 : ncols + c0 + cn],
                        start=False,
                        stop=True,
                    )
                    v0 = (h0 + 4 * ci - hbase) * WROW + 1
                    dst = AP(
                        tensor=spt,
                        offset=v0,
                        ap=[[SPLEN, BPC * 3], [WROW, nrows], [1, W]],
                    )
                    src = ps[:, :nrows, :]
                    if ci % 2 == 0:
                        nc.vector.tensor_copy(dst, src)
                    else:
                        nc.scalar.copy(dst, src)

                # build P9 single-hop: one DMA per (i,jj), both batches at
                # once (dst partitions 3i+jj and 9+3i+jj, stride 9).
                # P9[b*9+3i+jj, u] = sp[b*3+i, i... shifted]:
                #   = xp_b[32s*130 + u + jj*130 + (2-i)]
                spt_ = sp.tensor
                p9t = p9.tensor
                dmae = [nc.gpsimd, nc.gpsimd, nc.scalar]
                for i in range(3):
                    for jj in range(3):
                        m = 3 * i + jj
                        dmae[m % 3].dma_start(
                            out=AP(
                                tensor=p9t,
                                offset=m * PWIN,
                                ap=[[9 * PWIN, BPC], [1, PWIN]],
                            ),
                            in_=AP(
                                tensor=spt_,
                                offset=i * SPLEN + jj * WROW + 2 - i,
                                ap=[[3 * SPLEN, BPC], [1, PWIN]],
                            ),
                            single_packet=True,
                        )
                return p9

            def emit_warm():
                # dep-free matmuls that the PE chews on while waiting for a
                # P9 chain; keeps the HAM clock-gate at full rate.
                for _ in range(6):
                    ps = cs_psum.tile([BPC * 3, 4, W], f32, tag="cs")
                    nc.tensor.matmul(
                        ps[:, :, :],
                        ones_t[:, :],
                        xins[0][:, 0:512],
                        start=True,
                        stop=True,
                    )

            def emit_cv_and_out(s, p9):
                # conv: one K=20 fp32r matmul per 3-row chunk + psum->yt->hbm
                yt = y_pool.tile([NOUT, SH, W], f32, tag="yout")
                nchunk = (SH + 2) // 3
                for c in range(nchunk):
                    rr0 = c * 3
                    nrr = min(3, SH - rr0)
                    nn = nrr * WROW
                    ps = cv_psum.tile([NOUT, 3, WROW], f32, tag="cv")
                    nc.tensor.matmul(
                        ps[:, :nrr, :],
                        wb_t[:, :],
                        p9[:, rr0 * WROW : rr0 * WROW + nn],
                        start=True,
                        stop=True,
                    )
                    if c % 2 == 0:
                        nc.vector.tensor_copy(
                            yt[:, rr0 : rr0 + nrr, :], ps[:, :nrr, 0:W]
                        )
                    else:
                        nc.scalar.copy(yt[:, rr0 : rr0 + nrr, :], ps[:, :nrr, 0:W])

                half = SH // 2
                nc.sync.dma_start(
                    out=y.ap()[:, SH * s * W : (SH * s + half) * W],
                    in_=yt[:, :half, :],
                )
                nc.sync.dma_start(
                    out=y.ap()[:, (SH * s + half) * W : SH * (s + 1) * W],
                    in_=yt[:, half:, :],
                )

            # software-pipelined emission, two cs-stages ahead: PE stream is
            # cs0 cs1 cs2 cv0 cs3 cv1 cv2 cv3 so conv never heads the queue
            # while its P-build chain is still in flight.  Input DMAs are
            # emitted one slice ahead so they never queue behind P-chain
            # waits on their engine.
            DEPTH = 2
            p9s = {}
            xins = {s: emit_in(s) for s in range(NS)}
            for s in range(NS + DEPTH):
                if s < NS:
                    p9s[s] = emit_cs_and_p(s, xins[s])
                if s >= DEPTH:
                    emit_warm()
                    emit_cv_and_out(s - DEPTH, p9s[s - DEPTH])

    nc.compile()
    return nc


def _host_prep(x, weight, bias):
    bf = ml_dtypes.bfloat16
    wsum = weight.sum(axis=1)  # [COUT, 3, 3]
    wb = np.zeros((BPC * 9 + 1, NOUT), np.float32)
    for b in range(BPC):
        for i in range(3):
            for jj in range(3):
                wb[b * 9 + i * 3 + jj, b * COUT : (b + 1) * COUT] = wsum[
                    :, 2 - jj, i
                ]
    wb[BPC * 9, :] = np.tile(bias, BPC)
    ones_cs = np.zeros((NPART, BPC * 3), np.float32)
    for b in range(BPC):
        ones_cs[b * CIN : (b + 1) * CIN, b * 3 : (b + 1) * 3] = 1.0
    ones_cs = ones_cs.astype(bf)
    ones_p = np.ones((1, PWIN), np.float32)

    in_maps = []
    for r in range(N_CORES):
        xs = np.ascontiguousarray(
            x[r * BPC : (r + 1) * BPC].reshape(NPART, H, W)
        ).astype(np.float32)
        xhi = xs.astype(bf)
        xlo = (xs - xhi.astype(np.float32)).astype(bf)
        xpack = np.empty((NPART, XPACK_LEN), dtype=bf)
        for s in range(NS):
            h0, he = _slice_rows(s)
            n = (he - h0) * W
            o = _SLICE_OFF[s]
            xpack[:, o : o + n] = xhi[:, h0:he].reshape(NPART, n)
            xpack[:, o + n : o + 2 * n] = xlo[:, h0:he].reshape(NPART, n)
        in_maps.append(
            {
                "xpack": xpack,
                "ones_cs": ones_cs,
                "wb": wb,
                "ones_p": ones_p,
            }
        )
    return in_maps


def kernel(x, weight, bias):
    from concourse.bass_utils import run_bass_kernel_spmd

    x = np.asarray(x)
    weight = np.asarray(weight)
    bias = np.asarray(bias)
    nc = _build()
    in_maps = _host_prep(x, weight, bias)
    res = run_bass_kernel_spmd(nc, in_maps, core_ids=list(range(N_CORES)))
    out = np.concatenate(
        [
            res.results[r]["y"].reshape(BPC, COUT, H, W)
            for r in range(N_CORES)
        ],
        axis=0,
    )
    return out.astype(np.float32)



# revision 44
# speedup vs baseline: 2.4044x; 2.4044x over previous
"""FFTConv2d kernel for trn2, 8 NeuronCores.

Math: reference einsum 'bchw,oihw->bohw' factorizes:
  Y[b,o] = conv_full(sum_c x[b,c], sum_i w[o,i])[1:-1,1:-1] + bias[o]
i.e. a single-channel 3x3 "same" convolution (flipped kernel) per (b,o).

Per core (2 batches), all-bf16 dataflow (gate is rel_err < 2e-2; bf16
rounding of x, P9, weights and y contributes ~3e-3 combined):
  1. DMA x slice in as bf16 (natural [b*c, h*w] layout) in ~2k-column
     pieces; partitions=(b,c).
  2. Channel-sum via PE matmul with ones-indicator lhsT -> PSUM [6, n]
     (3 replicated copies per batch).
  3. Copy PSUM -> padded staging SBUF [6, (sh+2)*130] bf16 (row stride
     130, zero borders), alternating vector/scalar engines.
  4. Build P9 [19, pwin] bf16: partition (b, 3i+jj) = staging copy i
     col-shifted by jj*130 + (2-i); one contiguous DMA per tap,
     alternating gpsimd (SWDGE) / sync (HWDGE) queues.  Partition 18
     holds ones (bias).
  5. Conv: per 3-row output chunk, one K=19 bf16 matmul (taps via P9
     partitions) -> PSUM [128, 3, 130]; all (b,o) images at once; bias
     rides the ones row.
  6. Copy PSUM -> Y SBUF as bf16 (dropping the 2 pad columns per
     130-row), DMA out in row-chunk-aligned pieces; host upcasts.

Slices are ASYMMETRIC (default 80+48 rows): the tail chain (P9 build ->
conv -> copies -> yout) of the final slice bounds the kernel end, so the
last slice is small while the first one streams behind the input DMA.
"""

import os
import sys
from functools import lru_cache

import numpy as np

for _p in ("/opt/trn_rl_repo", "/root/.axon_site/_ro/trn_rl_repo"):
    if os.path.isdir(_p) and _p not in sys.path:
        sys.path.insert(0, _p)

import ml_dtypes

B, CIN, COUT, H, W = 16, 64, 64, 128, 128
N_CORES = 8
BPC = B // N_CORES  # batches per core = 2
WROW = W + 2  # padded row stride = 130
NPART = BPC * CIN  # 128 input partitions (b, c)
NOUT = BPC * COUT  # 128 output partitions (b, o)

SLICE_ROWS = [int(t) for t in os.environ.get("K_SLICES", "72,56").split(",")]
assert sum(SLICE_ROWS) == H
NS = len(SLICE_ROWS)
SLICE_START = [sum(SLICE_ROWS[:i]) for i in range(NS)]
PWIN_MAX = max(SLICE_ROWS) * WROW


def _slice_geom(s):
    start = SLICE_START[s]
    sh = SLICE_ROWS[s]
    h0 = max(0, start - 1)
    he = min(H, start + sh + 1)
    pwin = sh * WROW
    splen = (sh + 2) * WROW + 2
    return start, sh, h0, he, pwin, splen


@lru_cache(maxsize=1)
def _build():
    import concourse.bacc as bacc
    import concourse.mybir as mybir
    import concourse.tile as tile
    from concourse.ap import AP

    f32 = mybir.dt.float32
    bf16 = mybir.dt.bfloat16

    nc = bacc.Bacc("TRN2", target_bir_lowering=False, debug=False, num_devices=N_CORES)

    xp = nc.dram_tensor("xpack", [NPART, H * W], bf16, kind="ExternalInput")
    ones_cs = nc.dram_tensor("ones_cs", [NPART, BPC * 3], bf16, kind="ExternalInput")
    wb = nc.dram_tensor("wb", [BPC * 9 + 1, NOUT], bf16, kind="ExternalInput")
    ones_p = nc.dram_tensor("ones_p", [1, PWIN_MAX], bf16, kind="ExternalInput")
    y = nc.dram_tensor("y", [NOUT, H * W], bf16, kind="ExternalOutput")

    with tile.TileContext(nc) as tc:
        with (
            tc.tile_pool(name="xin", bufs=2) as xin_pool,
            tc.tile_pool(name="sp", bufs=1) as sp_pool,
            tc.tile_pool(name="pbuf", bufs=1) as p_pool,
            tc.tile_pool(name="yout", bufs=2) as y_pool,
            tc.tile_pool(name="consts", bufs=1) as c_pool,
            tc.tile_pool(
                name="cs_ps", bufs=int(os.environ.get("K_CSBUF", "4")),
                space="PSUM",
            ) as cs_psum,
            tc.tile_pool(
                name="cv_ps", bufs=int(os.environ.get("K_CVBUF", "4")),
                space="PSUM",
            ) as cv_psum,
        ):
            ones_t = c_pool.tile([NPART, BPC * 3], bf16, tag="ones_cs")
            wb_t = c_pool.tile([BPC * 9 + 1, NOUT], bf16, tag="wb")

            # per-slice staging + P9 buffers (zero borders persist)
            spbufs = []
            p9bufs = []
            for s in range(NS):
                _, sh, _, _, pwin, splen = _slice_geom(s)
                sp = sp_pool.tile([BPC * 3, splen], bf16, tag=f"SP{s}")
                spt0 = sp.tensor
                nc.vector.memset(
                    AP(tensor=spt0, offset=WROW - 1,
                       ap=[[splen, BPC * 3], [WROW, sh + 2], [1, 2]]),
                    0.0,
                )
                nc.vector.memset(sp[:, 0:WROW], 0.0)
                nc.vector.memset(sp[:, splen - 1 : splen], 0.0)
                spbufs.append(sp)
                p9 = p_pool.tile([BPC * 9 + 1, pwin], bf16, tag=f"P9{s}")
                p9bufs.append(p9)

            # GPSIMD cannot read PSUM, so PSUM->SBUF copies alternate
            # between the vector (DVE) and scalar (Activation) engines.
            cpeng = [nc.vector.tensor_copy, nc.scalar.copy]

            def emit_in(s):
                _, sh, h0, he, _, _ = _slice_geom(s)
                ncols = (he - h0) * W
                xin = xin_pool.tile([NPART, (sh + 2) * W], bf16, tag="xin")
                o = h0 * W
                # pieces of ~2k columns so cs matmuls track the DMA stream;
                # the very first piece is small so the pipeline starts fast
                per_slice = os.environ.get("K_XPIECE", "1792").split("|")
                sizes = [int(t) for t in
                         per_slice[min(s, len(per_slice) - 1)].split(",")]
                first = [512] if s == 0 else []
                cuts = []
                pos = 0
                for sz in first + sizes * 16:
                    pos += sz
                    if pos >= ncols:
                        break
                    cuts.append(pos)
                for a0, a1 in zip([0] + cuts, cuts + [ncols]):
                    nc.sync.dma_start(
                        out=xin[:, a0:a1], in_=xp.ap()[:, o + a0 : o + a1]
                    )
                return xin

            def emit_p9_taps(s):
                start, sh, h0, he, pwin, splen = _slice_geom(s)
                sp = spbufs[s]
                spt = sp.tensor
                p9 = p9bufs[s]
                p9t = p9.tensor
                qmap = {"g": nc.gpsimd, "s": nc.sync, "v": nc.vector, "a": nc.scalar}
                rot = os.environ.get("K_P9Q", "sg")
                dmae = [qmap[c] for c in rot]
                for i in range(3):
                    for jj in range(3):
                        m = 3 * i + jj
                        dmae[m % len(dmae)].dma_start(
                            out=AP(
                                tensor=p9t,
                                offset=m * pwin,
                                ap=[[9 * pwin, BPC], [1, pwin]],
                            ),
                            in_=AP(
                                tensor=spt,
                                offset=i * splen + jj * WROW + 2 - i,
                                ap=[[3 * splen, BPC], [1, pwin]],
                            ),
                            single_packet=True,
                        )
                return p9

            def emit_cs_and_p(s, xin, p9_taps=True):
                start, sh, h0, he, pwin, splen = _slice_geom(s)
                hbase = start - 1  # staging v-row 0 = image row hbase
                ncols = (he - h0) * W
                sp = spbufs[s]
                spt = sp.tensor
                p9 = p9bufs[s]

                if s == NS - 1:
                    # bottom border: zero staging rows beyond image row 127
                    vz = (H - hbase) * WROW
                    nc.vector.memset(sp[:, vz:splen], 0.0)

                # channel sum: ones^T @ x, PSUM -> padded staging (bf16)
                nchunks = (ncols + 511) // 512
                for ci in range(nchunks):
                    c0 = ci * 512
                    cn = min(512, ncols - c0)
                    nrows = cn // W
                    ps = cs_psum.tile([BPC * 3, 4, W], f32, tag="cs")
                    nc.tensor.matmul(
                        ps[:, :nrows, :],
                        ones_t[:, :],
                        xin[:, c0 : c0 + cn],
                        start=True,
                        stop=True,
                    )
                    v0 = (h0 + 4 * ci - hbase) * WROW + 1
                    dst = AP(
                        tensor=spt,
                        offset=v0,
                        ap=[[splen, BPC * 3], [WROW, nrows], [1, W]],
                    )
                    src = ps[:, :nrows, :]
                    cpeng[ci % 2](dst, src)

                if p9_taps:
                    return emit_p9_taps(s)
                return p9bufs[s]

            def emit_warm():
                # dep-free matmuls that the PE chews on while waiting for a
                # P9 chain; keeps the HAM clock-gate at full rate.
                for _ in range(int(os.environ.get("K_WARM", "2"))):
                    ps = cs_psum.tile([BPC * 3, 4, W], f32, tag="cs")
                    nc.tensor.matmul(
                        ps[:, :, :],
                        ones_t[:, :],
                        xins[0][:, 0:512],
                        start=True,
                        stop=True,
                    )

            def emit_cv_and_out(s, p9):
                # conv: one K=19 bf16 matmul per 3-row chunk + psum->yt->hbm
                start, sh, h0, he, pwin, splen = _slice_geom(s)
                yt = y_pool.tile([NOUT, sh, W], bf16, tag="yout")
                ytt = yt.tensor
                CROWS = int(os.environ.get("K_CROWS", "3"))
                nchunk = (sh + CROWS - 1) // CROWS
                # row-chunk-aligned yout pieces, emitted as soon as their
                # rows are copied so the DMAs interleave with the copies
                # (flat AP: per-partition runs are contiguous)
                step = (
                    int(os.environ.get("K_YSTEP1", "15"))
                    if s == NS - 1
                    else int(os.environ.get("K_YSTEP0", "15"))
                )
                pieces = list(range(0, sh - 1, step)) + [sh]
                spans = list(zip(pieces[:-1], pieces[1:]))
                pi = 0

                yq = os.environ.get("K_YQ", "s")
                yqmap = {"s": nc.sync, "a": nc.scalar, "g": nc.gpsimd}

                def flush_pieces(done_rows):
                    nonlocal pi
                    while pi < len(spans) and spans[pi][1] <= done_rows:
                        r0, r1 = spans[pi]
                        yqmap[yq[pi % len(yq)]].dma_start(
                            out=y.ap()[:, (start + r0) * W : (start + r1) * W],
                            in_=AP(
                                tensor=ytt,
                                offset=r0 * W,
                                ap=[[sh * W, NOUT], [1, (r1 - r0) * W]],
                            ),
                        )
                        pi += 1

                for c in range(nchunk):
                    rr0 = c * CROWS
                    nrr = min(CROWS, sh - rr0)
                    nn = nrr * WROW
                    ps = cv_psum.tile([NOUT, CROWS, WROW], f32, tag="cv")
                    nc.tensor.matmul(
                        ps[:, :nrr, :],
                        wb_t[:, :],
                        p9[:, rr0 * WROW : rr0 * WROW + nn],
                        start=True,
                        stop=True,
                    )
                    cpeng[(c + 1) % 2](
                        yt[:, rr0 : rr0 + nrr, :], ps[:, :nrr, 0:W]
                    )
                    flush_pieces(rr0 + nrr)
                flush_pieces(sh)

            # software-pipelined emission: input DMAs first (nothing
            # queues ahead of them), consts next, then per-slice flows.
            DEPTH = 1
            p9s = {}
            xins = {s: emit_in(s) for s in range(NS)}
            nc.scalar.dma_start(out=ones_t[:, :], in_=ones_cs.ap()[:, :])
            nc.scalar.dma_start(out=wb_t[:, :], in_=wb.ap()[:, :])
            for s in range(NS):
                _, _, _, _, pwin, _ = _slice_geom(s)
                nc.gpsimd.dma_start(
                    out=p9bufs[s][BPC * 9 : BPC * 9 + 1, :],
                    in_=ones_p.ap()[0:1, :pwin],
                )
            for s in range(NS + DEPTH):
                if s < NS:
                    p9s[s] = emit_cs_and_p(s, xins[s])
                if s >= DEPTH:
                    emit_warm()
                    emit_cv_and_out(s - DEPTH, p9s[s - DEPTH])

    nc.compile()
    return nc


def _host_prep(x, weight, bias):
    bf = ml_dtypes.bfloat16
    wsum = weight.sum(axis=1)  # [COUT, 3, 3]
    wb = np.zeros((BPC * 9 + 1, NOUT), np.float32)
    for b in range(BPC):
        for i in range(3):
            for jj in range(3):
                wb[b * 9 + i * 3 + jj, b * COUT : (b + 1) * COUT] = wsum[
                    :, 2 - jj, i
                ]
    wb[BPC * 9, :] = np.tile(bias, BPC)
    wb = wb.astype(bf)
    ones_cs = np.zeros((NPART, BPC * 3), np.float32)
    for b in range(BPC):
        ones_cs[b * CIN : (b + 1) * CIN, b * 3 : (b + 1) * 3] = 1.0
    ones_cs = ones_cs.astype(bf)
    ones_p = np.ones((1, PWIN_MAX), bf)

    in_maps = []
    for r in range(N_CORES):
        xpack = np.ascontiguousarray(
            x[r * BPC : (r + 1) * BPC].reshape(NPART, H * W)
        ).astype(bf)
        in_maps.append(
            {
                "xpack": xpack,
                "ones_cs": ones_cs,
                "wb": wb,
                "ones_p": ones_p,
            }
        )
    return in_maps


def kernel(x, weight, bias):
    from concourse.bass_utils import run_bass_kernel_spmd

    x = np.asarray(x)
    weight = np.asarray(weight)
    bias = np.asarray(bias)
    nc = _build()
    in_maps = _host_prep(x, weight, bias)
    res = run_bass_kernel_spmd(nc, in_maps, core_ids=list(range(N_CORES)))
    out = np.concatenate(
        [
            np.asarray(res.results[r]["y"]).astype(np.float32).reshape(
                BPC, COUT, H, W
            )
            for r in range(N_CORES)
        ],
        axis=0,
    )
    return out
